# revision 24
# baseline (speedup 1.0000x reference)
"""SkipGram negative-sampling loss kernel for 8 Trainium2 NeuronCores.

Strategy: data-parallel over walks (batch). The 1M x 128 embedding table is
replicated to every core's HBM; each core handles B/8 = 128 walks:
  - per-column indirect-DMA gathers (128 rows / instruction) of the walk and
    neg embeddings into SBUF, laid out [walk -> partition, position*D -> free]
  - cast to bf16 (2x DVE mode), dot products via elementwise mult +
    pairwise-halving add + segmented reduce
  - softplus via ScalarE Exp then Ln(x+1) with per-partition accumulate
    (accum_out accumulates on HW; zeroed once)
  - each core returns [128, 1] partial sums; host sums and divides.
"""

import sys
import types

import numpy as np
import ml_dtypes

try:  # missing in some containers; shim so trace=True degrades gracefully
    from antenv.axon_hooks import get_axon_ntff_profile_hook  # noqa: F401
except Exception:
    _m = types.ModuleType("antenv.axon_hooks")
    _m.get_axon_ntff_profile_hook = lambda: None
    sys.modules["antenv.axon_hooks"] = _m

import concourse.bass as bass
import concourse.bacc as bacc
import concourse.tile as tile
import concourse.mybir as mybir
from concourse.bass_utils import run_bass_kernel_spmd

F32 = mybir.dt.float32
BF16 = mybir.dt.bfloat16
I32 = mybir.dt.int32

N_CORES = 8


def build_kernel(n_walks, L, A, NEG, D, n_nodes, n_cores=N_CORES, reps=1, debug=False):
    """Build the SPMD Bass module (same NEFF on every core).

    reps > 1 repeats the whole workload (for slope timing); rep r writes its
    partials to out[:, r].
    """
    W1 = L - A  # window_size - 1 = number of pos offsets (4)
    H = D // 2
    nc = bacc.Bacc(
        "TRN2",
        target_bir_lowering=False,
        debug=False,
        num_devices=n_cores,
        num_swdge_queues=4,
    )
    walk_idx = nc.dram_tensor("walk_idx", [n_walks, L], I32, kind="ExternalInput")
    neg_idx = nc.dram_tensor("neg_idx", [n_walks, NEG * A], I32, kind="ExternalInput")
    embed = nc.dram_tensor("embed", [n_nodes, D], F32, kind="ExternalInput")
    out = nc.dram_tensor("out", [n_walks, reps], F32, kind="ExternalOutput")
    NPdbg = (L - A) + NEG
    if debug:
        ew_o = nc.dram_tensor("ew_o", [n_walks, L * D], F32, kind="ExternalOutput")
        en_o = nc.dram_tensor("en_o", [n_walks, A * D], F32, kind="ExternalOutput")
        logit_o = nc.dram_tensor(
            "logit_o", [n_walks, NPdbg * A], F32, kind="ExternalOutput"
        )
        sp_o = nc.dram_tensor("sp_o", [n_walks, NPdbg * A], F32, kind="ExternalOutput")

    with tile.TileContext(nc) as tc:
        with (
            tc.tile_pool(name="idx", bufs=1) as idxp,
            tc.tile_pool(name="ew16", bufs=1) as ew16p,
            tc.tile_pool(name="en16", bufs=2) as en16p,
            tc.tile_pool(name="prod", bufs=2) as prodp,
            tc.tile_pool(name="half", bufs=1) as halfp,
            tc.tile_pool(name="small", bufs=2) as smallp,
            tc.tile_pool(name="accp", bufs=1) as accp,
        ):
            wi = idxp.tile([n_walks, L], I32)
            nc.sync.dma_start(out=wi[:], in_=walk_idx[:])
            ni = idxp.tile([n_walks, NEG * A], I32)
            nc.sync.dma_start(out=ni[:], in_=neg_idx[:])

            NP = W1 + NEG  # planes per rep
            acc = accp.tile([n_walks, reps * NP], F32)

            qrr = [0]

            def gather_cast(idx_ap, ncols, out16_ap):
                """column-wise gather of ncols rows/partition, f32 -> bf16
                cast in the SDMA datapath (SWDGE cast; HBM reads stay 512B
                line-rate, only the SBUF write side is 256B which has no
                small-descriptor penalty).

                Gathers are spread round-robin over the 4 SWDGE queues so
                descriptor generation runs on all 4 Q7 core pairs."""
                for k in range(ncols):
                    inst = nc.gpsimd.indirect_dma_start(
                        out=out16_ap[:, k * D : (k + 1) * D],
                        out_offset=None,
                        in_=embed[:],
                        in_offset=bass.IndirectOffsetOnAxis(
                            ap=idx_ap[:, k : k + 1], axis=0
                        ),
                    )
                    q = qrr[0] % 4
                    qrr[0] += 1
                    inst.queue = f"qPoolDynamic{q or ''}"

            for r in range(reps):
                ew16 = ew16p.tile([n_walks, L * D], BF16)
                gather_cast(wi, L, ew16[:])
                plane_ctr = [0]

                def dot_softplus_accum(other16_ap, sp_scale):
                    prod = prodp.tile([n_walks, A * D], BF16)
                    nc.vector.tensor_mul(prod[:], ew16[:, 0 : A * D], other16_ap)
                    p3 = prod[:].rearrange("p (a d) -> p a d", d=D)
                    half = halfp.tile([n_walks, A * H], BF16)
                    h3 = half[:].rearrange("p (a d) -> p a d", d=H)
                    nc.vector.tensor_add(h3, p3[:, :, 0:H], p3[:, :, H:D])
                    logit = smallp.tile([n_walks, A], F32)
                    nc.vector.tensor_reduce(
                        logit[:],
                        h3,
                        axis=mybir.AxisListType.X,
                        op=mybir.AluOpType.add,
                    )
                    # stable softplus(s*x) = max(s*x,0) + ln(1 + exp(-|x|));
                    # naive ln(exp(x)+1) breaks on HW act tables for |x|>~50
                    ab = smallp.tile([n_walks, A], F32)
                    nc.scalar.activation(
                        ab[:], logit[:], mybir.ActivationFunctionType.Abs
                    )
                    e = smallp.tile([n_walks, A], F32)
                    nc.scalar.activation(
                        e[:], ab[:], mybir.ActivationFunctionType.Exp, scale=-1.0
                    )
                    ln1 = smallp.tile([n_walks, A], F32)
                    nc.scalar.activation(
                        ln1[:], e[:], mybir.ActivationFunctionType.Ln, bias=1.0
                    )
                    rl = smallp.tile([n_walks, A], F32)
                    nc.vector.tensor_scalar(
                        rl[:],
                        logit[:],
                        sp_scale,
                        0.0,
                        mybir.AluOpType.mult,
                        mybir.AluOpType.max,
                    )
                    sp = smallp.tile([n_walks, A], F32)
                    col = r * NP + plane_ctr[0]
                    plane_ctr[0] += 1
                    nc.vector.tensor_add(sp[:], ln1[:], rl[:])
                    nc.vector.tensor_reduce(
                        acc[:, col : col + 1],
                        sp[:],
                        axis=mybir.AxisListType.X,
                        op=mybir.AluOpType.add,
                    )
                    if debug and r == 0:
                        pidx = plane_ctr[0] - 1
                        nc.sync.dma_start(
                            out=logit_o[:, pidx * A : (pidx + 1) * A], in_=logit[:]
                        )
                        nc.sync.dma_start(
                            out=sp_o[:, pidx * A : (pidx + 1) * A], in_=sp[:]
                        )

                for i in range(1, W1 + 1):
                    dot_softplus_accum(ew16[:, i * D : (i + A) * D], -1.0)

                for j in range(NEG):
                    en16 = en16p.tile([n_walks, A * D], BF16)
                    gather_cast(ni[:, j * A : (j + 1) * A], A, en16[:])
                    dot_softplus_accum(en16[:], 1.0)

            # per-rep sum of the NP plane columns -> out[:, r]
            osum = accp.tile([n_walks, reps], F32)
            nc.vector.tensor_reduce(
                osum[:],
                acc[:].rearrange("p (r n) -> p r n", n=NP),
                axis=mybir.AxisListType.X,
                op=mybir.AluOpType.add,
            )
            nc.sync.dma_start(out=out[:], in_=osum[:])

    nc.compile()
    return nc


_NC_CACHE = {}


def _get_nc(key):
    if key not in _NC_CACHE:
        _NC_CACHE[key] = build_kernel(*key)
    return _NC_CACHE[key]


def make_in_maps(walk, neg, embed, n_cores=N_CORES):
    B, L = walk.shape
    A, NEG = neg.shape[1], neg.shape[2]
    nw = B // n_cores
    embed_f = np.ascontiguousarray(embed.astype(np.float32, copy=False))
    in_maps = []
    for c in range(n_cores):
        sl = slice(c * nw, (c + 1) * nw)
        wslice = np.ascontiguousarray(walk[sl].astype(np.int32, copy=False))
        # neg [nw, A, NEG] -> plane-major [nw, NEG*A]
        nslice = np.ascontiguousarray(
            neg[sl].astype(np.int32, copy=False).transpose(0, 2, 1).reshape(nw, NEG * A)
        )
        in_maps.append({"walk_idx": wslice, "neg_idx": nslice, "embed": embed_f})
    return in_maps


def kernel(walk, neg, embed, _trace=False):
    walk = np.asarray(walk)
    neg = np.asarray(neg)
    embed = np.asarray(embed)
    B, L = walk.shape
    A, NEG = neg.shape[1], neg.shape[2]
    n_nodes, D = embed.shape
    nw = B // N_CORES

    nc = _get_nc((nw, L, A, NEG, D, n_nodes, N_CORES))
    in_maps = make_in_maps(walk, neg, embed)
    res = run_bass_kernel_spmd(
        nc, in_maps, core_ids=list(range(N_CORES)), trace=_trace
    )
    total = 2 * B * A * NEG
    s = sum(r["out"][:, 0].astype(np.float64).sum() for r in res.results)
    loss = np.float32(s / total)
    if _trace:
        return loss, res
    return loss

